# revision 22
# baseline (speedup 1.0000x reference)
import sys

sys.path.insert(0, "/opt/trn_rl_repo")

import numpy as np
from contextlib import ExitStack
from concourse import bacc, bass_utils, tile, mybir

F32 = mybir.dt.float32
F32R = mybir.dt.float32r
BF16 = mybir.dt.bfloat16
TANH = mybir.ActivationFunctionType.Tanh

NCORES = 8


def _r11(x):
    # round to fp32r (11 explicit mantissa bits) so device-side fp32r
    # rounding of these constants is an exact identity
    i = np.ascontiguousarray(x, dtype=np.float32).view(np.int32)
    i = (i + 0x800) & ~0xFFF
    return i.view(np.float32)


def _rbf(x):
    # round fp32 to bf16 (round-to-nearest-even), kept in fp32 storage
    i = np.ascontiguousarray(x, dtype=np.float32).view(np.uint32)
    i = (i + 0x7FFF + ((i >> 16) & 1)) & 0xFFFF0000
    return i.view(np.float32)


def _build(nu, d_idx, steps, ncores):
    # Per-core layout: N = 16384 samples in 16 chunks of 512 cols
    # (2 samples/col). y state lives permanently in PSUM fp32 (banks
    # 0-3): chunk c -> partition strip 32*(c%4)+[0..8), cols
    # 512*(c//4)+[0..512). Rows within a strip: p = 4*dup + 2*AB + c
    # (dup duplicates so one K=8 mm1 applies W1hi+W1lo). Each Euler
    # step, mm2 (bf16, col-tiled at 32*(c%4), hi+lo weight pair)
    # ACCUMULATES dt*(W2 h + b2) onto y via start=False, so no vector
    # Euler add is needed. The state recirculates PSUM->SBUF (f32r)
    # via DVE/ACT copies (DMA and gpsimd can't read PSUM). mm1 stays
    # f32r (its dst is at partition 0; f32r can't col-tile) and is
    # 4-way row-tiled at 32*(c%4).
    nc = bacc.Bacc(
        "TRN2",
        target_bir_lowering=False,
        debug=False,
        enable_asserts=False,
        num_devices=ncores,
    )
    W = 8192  # 16 chunks * 512
    w1rep_d = nc.dram_tensor("w1rep", [128, 100], F32, kind="ExternalInput")
    b1bd_d = nc.dram_tensor("b1bd", [100, 1], F32, kind="ExternalInput")
    w2f_d = nc.dram_tensor("w2f", [101, 8 * nu], F32, kind="ExternalInput")
    eye12_d = nc.dram_tensor("eye12", [12, 8], F32, kind="ExternalInput")
    y0pre_d = nc.dram_tensor("y0pre", [12, W], F32, kind="ExternalInput")
    out_d = nc.dram_tensor("out", [steps, 4, W], BF16, kind="ExternalOutput")

    HB = 4
    with tile.TileContext(nc) as tc:
        with ExitStack() as ctx:
            sb = ctx.enter_context(tc.tile_pool(name="sb", bufs=1, space="SBUF"))
            ps = ctx.enter_context(tc.tile_pool(name="ps", bufs=1, space="PSUM"))

            w1_sb = sb.tile([128, 100], BF16, tag="w1", name="w1_sb")
            b1_sb = sb.tile([100, 1], F32, tag="b1", name="b1_sb")
            w2_sb = sb.tile([101, 8 * nu], BF16, tag="w2", name="w2_sb")
            eye_sb = sb.tile([12, 8], BF16, tag="eye", name="eye_sb")
            y0_sb = sb.tile([12, W], BF16, tag="y0", name="y0_sb")
            st_w1 = sb.tile([128, 100], F32, tag="sw1", name="st_w1")
            st_w2 = sb.tile([101, 8 * nu], F32, tag="sw2", name="st_w2")
            st_eye = sb.tile([12, 8], F32, tag="sey", name="st_eye")
            st_y0 = sb.tile([12, W], F32, tag="sy0", name="st_y0")
            yr = sb.tile([128, 2048], BF16, tag="yr", name="yr")
            h_bufs = [
                sb.tile([101, 1024], BF16, tag=f"h{i}", name=f"h{i}")
                for i in range(HB)
            ]
            y_ps = ps.tile([128, 2048], F32, tag="y", name="y_ps")
            a_bufs = [
                ps.tile([128, 1024], F32, tag=f"a{i}", name=f"a{i}") for i in range(2)
            ]

            nc.sync.dma_start(out=st_w1[:, :], in_=w1rep_d[:, :])
            nc.sync.dma_start(out=b1_sb[:, :], in_=b1bd_d[:, :])
            nc.sync.dma_start(out=st_w2[:, :], in_=w2f_d[:, :])
            nc.sync.dma_start(out=st_eye[:, :], in_=eye12_d[:, :])
            nc.sync.dma_start(out=st_y0[:, :], in_=y0pre_d[:, :])
            nc.vector.tensor_copy(out=w1_sb[:, :], in_=st_w1[:, :])
            nc.vector.tensor_copy(out=w2_sb[:, :], in_=st_w2[:, :])
            nc.vector.tensor_copy(out=eye_sb[:, :], in_=st_eye[:, :])
            nc.vector.tensor_copy(out=y0_sb[:, :], in_=st_y0[:, :])

            # h row 100 is a constant-1 bias row (so mm2 adds dt*b2 via
            # lhsT row 100). memset can't write BF16 reliably and engine
            # partition bases must be 32-aligned, so stage rows 96-101 in
            # F32 and copy; rows 96-99 get overwritten by every tanh.
            ones_st = sb.tile([101, 1024], F32, tag="one", name="ones_st")
            nc.vector.memset(ones_st[96:101, :], 1.0)
            for i in range(HB):
                nc.vector.tensor_copy(
                    out=h_bufs[i][96:101, :], in_=ones_st[96:101, :]
                )

            # init: y_psum[chunk] = y0hi + y0mid + y0lo via identity matmul
            for c in range(16):
                j, b = c % 4, c // 4
                nc.tensor.matmul(
                    y_ps[32 * j : 32 * j + 8, 512 * b : 512 * b + 512],
                    lhsT=eye_sb[:, :],
                    rhs=y0_sb[:, 512 * c : 512 * c + 512],
                    start=True,
                    stop=True,
                    tile_position=(0, 32 * j),
                )

            def recirc(j, b, eng=None):
                # PSUM y (fp32) -> SBUF yr (bf16) for the next step's mm1;
                # DMA/gpsimd can't read PSUM so this is DVE (mostly) work.
                # Per-(strip, bank) pieces emitted right after each bank's
                # mm2 batch give the next step's rounds 5+ rounds of slack.
                cs = slice(512 * b, 512 * b + 512)
                dst = yr[32 * j : 32 * j + 8, cs]
                src = y_ps[32 * j : 32 * j + 8, cs]
                if eng == "act":
                    nc.scalar.copy(out=dst, in_=src)
                else:
                    nc.vector.tensor_copy(out=dst, in_=src)

            def mm1(r, u):
                # global round r: step r//8, local round k=r%8 covers local
                # chunks {2k, 2k+1}; chunk c -> strip j=c%4, bank b=c//4.
                # Consecutive rounds use disjoint a tiles so the tanh WAR
                # chain spans 2 rounds (slack for sem hops).
                c = 2 * (r % 8) + u
                j, b = c % 4, c // 4
                nc.tensor.matmul(
                    a_bufs[r % 2][0:100, 512 * u : 512 * u + 512],
                    lhsT=w1_sb[32 * j : 32 * j + 8, :],
                    rhs=yr[32 * j : 32 * j + 8, 512 * b : 512 * b + 512],
                    start=True,
                    stop=True,
                    tile_position=(32 * j, 0),
                )

            def tanh(r):
                h = h_bufs[r % HB]
                nc.scalar.activation(
                    h[0:100, :],
                    a_bufs[r % 2][0:100, :],
                    TANH,
                    bias=b1_sb[:, :],
                )

            def mm2(s, c, di):
                j, b = c % 4, c // 4
                h = h_bufs[(8 * s + c // 2) % HB]
                hs = h[:, 512 * (c % 2) : 512 * (c % 2) + 512]
                nc.tensor.matmul(
                    y_ps[32 * j : 32 * j + 8, 512 * b : 512 * b + 512],
                    lhsT=w2_sb[:, 8 * di : 8 * di + 8],
                    rhs=hs,
                    start=False,
                    stop=True,
                    tile_position=(0, 32 * j),
                    skip_group_check=True,
                )

            def outdma(s, j, ph):
                # out[s] col 512*(4b+j)+i <- yr[2AB+c at 32j, 512b+i]
                nc.sync.dma_start(
                    out=out_d[s].rearrange(
                        "p (b j2 i) -> p b j2 i", b=4, j2=4, i=512
                    )[:, 2 * ph : 2 * ph + 2, j],
                    in_=yr[32 * j : 32 * j + 4, 1024 * ph : 1024 * ph + 1024]
                    .rearrange("p (b i) -> p b i", b=2, i=512),
                )

            for j in range(4):
                for b in range(4):
                    recirc(j, b)

            # Software-pipelined rounds of 2 chunks: even rounds hit strips
            # {0,1}, odd {2,3}; consecutive rounds use disjoint a tiles so
            # the tanh->mm1 WAR chain spans 2 rounds (slack for sem hops).
            # mm2s batch 4-way (4 distinct col strips) after each odd
            # round's tanh; recirc/out DMA per phase overlap 4+ rounds.
            R = steps * 8
            for r in range(R + 1):
                if r < R:
                    mm1(r, 0)
                    mm1(r, 1)
                if r >= 2 and (r - 1) % 8 % 2 == 1:
                    ro = r - 1  # odd round whose tanh is already emitted
                    s, k8 = ro // 8, ro % 8
                    di = d_idx[s]
                    for c in range(4 * (k8 // 2), 4 * (k8 // 2) + 4):
                        mm2(s, c, di)
                    b = k8 // 2  # this batch completed bank b
                    for j in range(4):
                        # last bank's strips 2,3 go to ACT (fills its
                        # boundary gap; their consumers are latest)
                        eng = "act" if (b == 3 and j >= 2) else None
                        recirc(j, b, eng)
                    if k8 == 3:
                        for j in range(4):
                            outdma(s, j, 0)
                    elif k8 == 7:
                        for j in range(4):
                            outdma(s, j, 1)
                if r < R:
                    tanh(r)
    nc.compile()
    return nc


def _prep(y0, t, w1, b1, w2, b2, ncores):
    B = y0.shape[0]
    steps = t.shape[0] - 1
    N = B // ncores
    dts = (t[1:] - t[:-1]).astype(np.float32)
    uniq, inv = np.unique(dts, return_inverse=True)
    nu = len(uniq)
    w1hi = _rbf(w1)
    w1lo = _rbf((w1 - w1hi).astype(np.float32))
    w1bd = np.zeros((8, 100), np.float32)
    w1bd[0:2, 0:50] = w1hi.T
    w1bd[2:4, 50:100] = w1hi.T
    w1bd[4:6, 0:50] = w1lo.T
    w1bd[6:8, 50:100] = w1lo.T
    w1rep = np.zeros((128, 100), np.float32)
    for j in range(4):
        w1rep[32 * j : 32 * j + 8] = w1bd
    b1bd = np.concatenate([b1, b1]).astype(np.float32).reshape(100, 1)
    w2f = np.zeros((101, 8 * nu), np.float32)
    for d in range(nu):
        dw2 = (uniq[d] * w2).astype(np.float32)
        db2 = (uniq[d] * b2).astype(np.float32)
        hi2 = _rbf(dw2)
        bhi = _rbf(db2)
        for dup in (0, 1):
            o = 8 * d + 4 * dup
            w2f[0:50, o : o + 2] = hi2.T
            w2f[50:100, o + 2 : o + 4] = hi2.T
        w2f[100, 8 * d : 8 * d + 8] = [bhi[0], bhi[1], bhi[0], bhi[1]] * 2
    eye12 = np.zeros((12, 8), np.float32)
    for r in range(12):
        for m in range(8):
            if r % 4 == m % 4:
                eye12[r, m] = 1.0
    y0hi = _rbf(y0)
    y0mid = _rbf((y0 - y0hi).astype(np.float32))
    y0lo = _rbf((y0 - y0hi - y0mid).astype(np.float32))
    in_maps = []
    for k in range(ncores):
        yk = np.empty((12, N // 2), np.float32)
        for src, base in ((y0hi, 0), (y0mid, 4), (y0lo, 8)):
            # row 2*AB+c, col 512*chunk+i = src[kN + 1024*chunk + 512*AB + i, c]
            blk = src[k * N : (k + 1) * N].reshape(16, 2, 512, 2)  # chunk,AB,i,c
            yk[base : base + 4] = (
                blk.transpose(1, 3, 0, 2).reshape(4, N // 2)
            )
        in_maps.append(
            {
                "w1rep": w1rep,
                "b1bd": b1bd,
                "w2f": w2f,
                "eye12": eye12,
                "y0pre": yk,
            }
        )
    return nu, list(inv), steps, N, in_maps


def run(y0, t, w1, b1, w2, b2, ncores=NCORES, steps_override=None, trace=False):
    y0 = np.ascontiguousarray(y0, dtype=np.float32)
    nu, inv, steps, N, in_maps = _prep(
        y0, np.asarray(t), np.asarray(w1), np.asarray(b1), np.asarray(w2),
        np.asarray(b2), ncores,
    )
    if steps_override is not None:
        steps = steps_override
    nc = _build(nu, inv, steps, ncores)
    res = bass_utils.run_bass_kernel_spmd(
        nc, in_maps, list(range(ncores)), trace=trace
    )
    B = y0.shape[0]
    out = np.empty((steps + 1, B, 2), np.float32)
    out[0] = y0
    for k in range(ncores):
        v = np.asarray(res.results[k]["out"]).astype(np.float32)
        v = v.reshape(steps, 2, 2, 16, 512)  # s, AB, c, chunk, i
        out[1:, k * N : (k + 1) * N, :] = (
            v.transpose(0, 3, 1, 4, 2).reshape(steps, N, 2)
        )
    return out, res


def kernel(**inputs):
    out, _ = run(
        inputs["y0"], inputs["t"], inputs["w1"], inputs["b1"], inputs["w2"],
        inputs["b2"],
    )
    return out


# revision 23
# speedup vs baseline: 1.2318x; 1.2318x over previous
import sys

sys.path.insert(0, "/opt/trn_rl_repo")

import numpy as np
from contextlib import ExitStack
from concourse import bacc, bass_utils, tile, mybir

F32 = mybir.dt.float32
F32R = mybir.dt.float32r
BF16 = mybir.dt.bfloat16
TANH = mybir.ActivationFunctionType.Tanh

NCORES = 8


def _r11(x):
    # round to fp32r (11 explicit mantissa bits) so device-side fp32r
    # rounding of these constants is an exact identity
    i = np.ascontiguousarray(x, dtype=np.float32).view(np.int32)
    i = (i + 0x800) & ~0xFFF
    return i.view(np.float32)


def _rbf(x):
    # round fp32 to bf16 (round-to-nearest-even), kept in fp32 storage
    i = np.ascontiguousarray(x, dtype=np.float32).view(np.uint32)
    i = (i + 0x7FFF + ((i >> 16) & 1)) & 0xFFFF0000
    return i.view(np.float32)


def _build(nu, d_idx, steps, ncores):
    # Per-core layout: N = 16384 samples in 16 chunks of 512 cols
    # (2 samples/col). y state lives permanently in PSUM fp32 (banks
    # 0-3): chunk c -> partition strip 32*(c%4)+[0..8), cols
    # 512*(c//4)+[0..512). Rows within a strip: p = 4*dup + 2*AB + c
    # (dup duplicates so one K=8 mm1 applies W1hi+W1lo). Each Euler
    # step, mm2 (bf16, col-tiled at 32*(c%4), hi+lo weight pair)
    # ACCUMULATES dt*(W2 h + b2) onto y via start=False, so no vector
    # Euler add is needed. The state recirculates PSUM->SBUF (f32r)
    # via DVE/ACT copies (DMA and gpsimd can't read PSUM). mm1 stays
    # f32r (its dst is at partition 0; f32r can't col-tile) and is
    # 4-way row-tiled at 32*(c%4).
    nc = bacc.Bacc(
        "TRN2",
        target_bir_lowering=False,
        debug=False,
        enable_asserts=False,
        num_devices=ncores,
    )
    W = 8192  # 16 chunks * 512
    w1rep_d = nc.dram_tensor("w1rep", [128, 100], F32, kind="ExternalInput")
    b1bd_d = nc.dram_tensor("b1bd", [100, 1], F32, kind="ExternalInput")
    w2f_d = nc.dram_tensor("w2f", [101, 8 * nu], F32, kind="ExternalInput")
    eye12_d = nc.dram_tensor("eye12", [12, 8], F32, kind="ExternalInput")
    y0pre_d = nc.dram_tensor("y0pre", [12, W], F32, kind="ExternalInput")
    out_d = nc.dram_tensor("out", [steps, 4, W], BF16, kind="ExternalOutput")

    HB = 4
    with tile.TileContext(nc) as tc:
        with ExitStack() as ctx:
            sb = ctx.enter_context(tc.tile_pool(name="sb", bufs=1, space="SBUF"))
            ps = ctx.enter_context(tc.tile_pool(name="ps", bufs=1, space="PSUM"))

            w1_sb = sb.tile([128, 100], BF16, tag="w1", name="w1_sb")
            b1_sb = sb.tile([100, 1], F32, tag="b1", name="b1_sb")
            w2_sb = sb.tile([101, 8 * nu], BF16, tag="w2", name="w2_sb")
            eye_sb = sb.tile([12, 8], BF16, tag="eye", name="eye_sb")
            y0_sb = sb.tile([12, W], BF16, tag="y0", name="y0_sb")
            st_w1 = sb.tile([128, 100], F32, tag="sw1", name="st_w1")
            st_w2 = sb.tile([101, 8 * nu], F32, tag="sw2", name="st_w2")
            st_eye = sb.tile([12, 8], F32, tag="sey", name="st_eye")
            st_y0 = sb.tile([12, W], F32, tag="sy0", name="st_y0")
            yr = sb.tile([128, 2048], BF16, tag="yr", name="yr")
            h_bufs = [
                sb.tile([101, 1024], BF16, tag=f"h{i}", name=f"h{i}")
                for i in range(HB)
            ]
            y_ps = ps.tile([128, 2048], F32, tag="y", name="y_ps")
            a_bufs = [
                ps.tile([128, 1024], F32, tag=f"a{i}", name=f"a{i}") for i in range(2)
            ]

            nc.sync.dma_start(out=st_w1[:, :], in_=w1rep_d[:, :])
            nc.sync.dma_start(out=b1_sb[:, :], in_=b1bd_d[:, :])
            nc.sync.dma_start(out=st_w2[:, :], in_=w2f_d[:, :])
            nc.sync.dma_start(out=st_eye[:, :], in_=eye12_d[:, :])
            nc.sync.dma_start(out=st_y0[:, :], in_=y0pre_d[:, :])
            nc.vector.tensor_copy(out=w1_sb[:, :], in_=st_w1[:, :])
            nc.vector.tensor_copy(out=w2_sb[:, :], in_=st_w2[:, :])
            nc.vector.tensor_copy(out=eye_sb[:, :], in_=st_eye[:, :])
            nc.vector.tensor_copy(out=y0_sb[:, :], in_=st_y0[:, :])

            # h row 100 is a constant-1 bias row (so mm2 adds dt*b2 via
            # lhsT row 100). memset can't write BF16 reliably and engine
            # partition bases must be 32-aligned, so stage rows 96-101 in
            # F32 and copy; rows 96-99 get overwritten by every tanh.
            ones_st = sb.tile([101, 1024], F32, tag="one", name="ones_st")
            nc.vector.memset(ones_st[96:101, :], 1.0)
            for i in range(HB):
                nc.vector.tensor_copy(
                    out=h_bufs[i][96:101, :], in_=ones_st[96:101, :]
                )

            # init: y_psum[chunk] = y0hi + y0mid + y0lo via identity matmul
            for c in range(16):
                j, b = c % 4, c // 4
                nc.tensor.matmul(
                    y_ps[32 * j : 32 * j + 8, 512 * b : 512 * b + 512],
                    lhsT=eye_sb[:, :],
                    rhs=y0_sb[:, 512 * c : 512 * c + 512],
                    start=True,
                    stop=True,
                    tile_position=(0, 32 * j),
                )

            def recirc(j, ph):
                # PSUM y (fp32) -> SBUF yr (f32r) for the next step's mm1;
                # DMA/gpsimd can't read PSUM so this is DVE work. Split per
                # (strip, phase) so it overlaps compute of the other phase.
                cs = slice(1024 * ph, 1024 * ph + 1024)
                nc.vector.tensor_copy(
                    out=yr[32 * j : 32 * j + 8, cs],
                    in_=y_ps[32 * j : 32 * j + 8, cs],
                )

            def mm1(r, u):
                # global round r: step r//8, local round k=r%8 covers local
                # chunks {2k, 2k+1}; chunk c -> strip j=c%4, bank b=c//4.
                # Consecutive rounds use disjoint a tiles so the tanh WAR
                # chain spans 2 rounds (slack for sem hops).
                c = 2 * (r % 8) + u
                j, b = c % 4, c // 4
                nc.tensor.matmul(
                    a_bufs[r % 2][0:100, 512 * u : 512 * u + 512],
                    lhsT=w1_sb[32 * j : 32 * j + 8, :],
                    rhs=yr[32 * j : 32 * j + 8, 512 * b : 512 * b + 512],
                    start=True,
                    stop=True,
                    tile_position=(32 * j, 0),
                )

            def tanh(r):
                h = h_bufs[r % HB]
                nc.scalar.activation(
                    h[0:100, :],
                    a_bufs[r % 2][0:100, :],
                    TANH,
                    bias=b1_sb[:, :],
                )

            def mm2(s, c, di):
                j, b = c % 4, c // 4
                h = h_bufs[(8 * s + c // 2) % HB]
                hs = h[:, 512 * (c % 2) : 512 * (c % 2) + 512]
                nc.tensor.matmul(
                    y_ps[32 * j : 32 * j + 8, 512 * b : 512 * b + 512],
                    lhsT=w2_sb[:, 8 * di : 8 * di + 8],
                    rhs=hs,
                    start=False,
                    stop=True,
                    tile_position=(0, 32 * j),
                    skip_group_check=True,
                )

            def outdma(s, j, ph):
                # out[s] col 512*(4b+j)+i <- yr[2AB+c at 32j, 512b+i]
                nc.sync.dma_start(
                    out=out_d[s].rearrange(
                        "p (b j2 i) -> p b j2 i", b=4, j2=4, i=512
                    )[:, 2 * ph : 2 * ph + 2, j],
                    in_=yr[32 * j : 32 * j + 4, 1024 * ph : 1024 * ph + 1024]
                    .rearrange("p (b i) -> p b i", b=2, i=512),
                )

            for j in range(4):
                recirc(j, 0)
                recirc(j, 1)

            # Software-pipelined rounds of 2 chunks: even rounds hit strips
            # {0,1}, odd {2,3}; consecutive rounds use disjoint a tiles so
            # the tanh->mm1 WAR chain spans 2 rounds (slack for sem hops).
            # mm2s batch 4-way (4 distinct col strips) after each odd
            # round's tanh; recirc/out DMA per phase overlap 4+ rounds.
            R = steps * 8
            for r in range(R + 1):
                if r < R:
                    mm1(r, 0)
                    mm1(r, 1)
                if r >= 2 and (r - 1) % 8 % 2 == 1:
                    ro = r - 1  # odd round whose tanh is already emitted
                    s, k8 = ro // 8, ro % 8
                    di = d_idx[s]
                    for c in range(4 * (k8 // 2), 4 * (k8 // 2) + 4):
                        mm2(s, c, di)
                    if k8 == 3:
                        for j in range(4):
                            recirc(j, 0)
                            outdma(s, j, 0)
                    elif k8 == 7:
                        for j in range(4):
                            recirc(j, 1)
                            outdma(s, j, 1)
                if r < R:
                    tanh(r)
    nc.compile()
    return nc


def _prep(y0, t, w1, b1, w2, b2, ncores):
    B = y0.shape[0]
    steps = t.shape[0] - 1
    N = B // ncores
    dts = (t[1:] - t[:-1]).astype(np.float32)
    uniq, inv = np.unique(dts, return_inverse=True)
    nu = len(uniq)
    w1hi = _rbf(w1)
    w1lo = _rbf((w1 - w1hi).astype(np.float32))
    w1bd = np.zeros((8, 100), np.float32)
    w1bd[0:2, 0:50] = w1hi.T
    w1bd[2:4, 50:100] = w1hi.T
    w1bd[4:6, 0:50] = w1lo.T
    w1bd[6:8, 50:100] = w1lo.T
    w1rep = np.zeros((128, 100), np.float32)
    for j in range(4):
        w1rep[32 * j : 32 * j + 8] = w1bd
    b1bd = np.concatenate([b1, b1]).astype(np.float32).reshape(100, 1)
    w2f = np.zeros((101, 8 * nu), np.float32)
    for d in range(nu):
        dw2 = (uniq[d] * w2).astype(np.float32)
        db2 = (uniq[d] * b2).astype(np.float32)
        hi2 = _rbf(dw2)
        bhi = _rbf(db2)
        for dup in (0, 1):
            o = 8 * d + 4 * dup
            w2f[0:50, o : o + 2] = hi2.T
            w2f[50:100, o + 2 : o + 4] = hi2.T
        w2f[100, 8 * d : 8 * d + 8] = [bhi[0], bhi[1], bhi[0], bhi[1]] * 2
    eye12 = np.zeros((12, 8), np.float32)
    for r in range(12):
        for m in range(8):
            if r % 4 == m % 4:
                eye12[r, m] = 1.0
    y0hi = _rbf(y0)
    y0mid = _rbf((y0 - y0hi).astype(np.float32))
    y0lo = _rbf((y0 - y0hi - y0mid).astype(np.float32))
    in_maps = []
    for k in range(ncores):
        yk = np.empty((12, N // 2), np.float32)
        for src, base in ((y0hi, 0), (y0mid, 4), (y0lo, 8)):
            # row 2*AB+c, col 512*chunk+i = src[kN + 1024*chunk + 512*AB + i, c]
            blk = src[k * N : (k + 1) * N].reshape(16, 2, 512, 2)  # chunk,AB,i,c
            yk[base : base + 4] = (
                blk.transpose(1, 3, 0, 2).reshape(4, N // 2)
            )
        in_maps.append(
            {
                "w1rep": w1rep,
                "b1bd": b1bd,
                "w2f": w2f,
                "eye12": eye12,
                "y0pre": yk,
            }
        )
    return nu, list(inv), steps, N, in_maps


def run(y0, t, w1, b1, w2, b2, ncores=NCORES, steps_override=None, trace=False):
    y0 = np.ascontiguousarray(y0, dtype=np.float32)
    nu, inv, steps, N, in_maps = _prep(
        y0, np.asarray(t), np.asarray(w1), np.asarray(b1), np.asarray(w2),
        np.asarray(b2), ncores,
    )
    if steps_override is not None:
        steps = steps_override
    nc = _build(nu, inv, steps, ncores)
    res = bass_utils.run_bass_kernel_spmd(
        nc, in_maps, list(range(ncores)), trace=trace
    )
    B = y0.shape[0]
    out = np.empty((steps + 1, B, 2), np.float32)
    out[0] = y0
    for k in range(ncores):
        v = np.asarray(res.results[k]["out"]).astype(np.float32)
        v = v.reshape(steps, 2, 2, 16, 512)  # s, AB, c, chunk, i
        out[1:, k * N : (k + 1) * N, :] = (
            v.transpose(0, 3, 1, 4, 2).reshape(steps, N, 2)
        )
    return out, res


def kernel(**inputs):
    out, _ = run(
        inputs["y0"], inputs["t"], inputs["w1"], inputs["b1"], inputs["w2"],
        inputs["b2"],
    )
    return out


# revision 25
# speedup vs baseline: 1.2402x; 1.0069x over previous
import sys

sys.path.insert(0, "/opt/trn_rl_repo")

import numpy as np
from contextlib import ExitStack
from concourse import bacc, bass_utils, tile, mybir

F32 = mybir.dt.float32
F32R = mybir.dt.float32r
BF16 = mybir.dt.bfloat16
TANH = mybir.ActivationFunctionType.Tanh

NCORES = 8


def _r11(x):
    # round to fp32r (11 explicit mantissa bits) so device-side fp32r
    # rounding of these constants is an exact identity
    i = np.ascontiguousarray(x, dtype=np.float32).view(np.int32)
    i = (i + 0x800) & ~0xFFF
    return i.view(np.float32)


def _rbf(x):
    # round fp32 to bf16 (round-to-nearest-even), kept in fp32 storage
    i = np.ascontiguousarray(x, dtype=np.float32).view(np.uint32)
    i = (i + 0x7FFF + ((i >> 16) & 1)) & 0xFFFF0000
    return i.view(np.float32)


def _build(nu, d_idx, steps, ncores):
    # Per-core layout: N = 16384 samples in 16 chunks of 512 cols
    # (2 samples/col). y state lives permanently in PSUM fp32 (banks
    # 0-3): chunk c -> partition strip 32*(c%4)+[0..8), cols
    # 512*(c//4)+[0..512). Rows within a strip: p = 4*dup + 2*AB + c
    # (dup duplicates so one K=8 mm1 applies W1hi+W1lo). Each Euler
    # step, mm2 (bf16, col-tiled at 32*(c%4), hi+lo weight pair)
    # ACCUMULATES dt*(W2 h + b2) onto y via start=False, so no vector
    # Euler add is needed. The state recirculates PSUM->SBUF (f32r)
    # via DVE/ACT copies (DMA and gpsimd can't read PSUM). mm1 stays
    # f32r (its dst is at partition 0; f32r can't col-tile) and is
    # 4-way row-tiled at 32*(c%4).
    nc = bacc.Bacc(
        "TRN2",
        target_bir_lowering=False,
        debug=False,
        enable_asserts=False,
        num_devices=ncores,
    )
    W = 8192  # 16 chunks * 512
    w1rep_d = nc.dram_tensor("w1rep", [128, 100], F32, kind="ExternalInput")
    b1bd_d = nc.dram_tensor("b1bd", [100, 1], F32, kind="ExternalInput")
    w2f_d = nc.dram_tensor("w2f", [101, 8 * nu], F32, kind="ExternalInput")
    eye12_d = nc.dram_tensor("eye12", [12, 8], F32, kind="ExternalInput")
    y0pre_d = nc.dram_tensor("y0pre", [12, W], F32, kind="ExternalInput")
    out_d = nc.dram_tensor("out", [steps, 4, W], BF16, kind="ExternalOutput")

    HB = 4
    with tile.TileContext(nc) as tc:
        with ExitStack() as ctx:
            sb = ctx.enter_context(tc.tile_pool(name="sb", bufs=1, space="SBUF"))
            ps = ctx.enter_context(tc.tile_pool(name="ps", bufs=1, space="PSUM"))

            w1_sb = sb.tile([128, 100], BF16, tag="w1", name="w1_sb")
            b1_sb = sb.tile([100, 1], F32, tag="b1", name="b1_sb")
            w2_sb = sb.tile([101, 8 * nu], BF16, tag="w2", name="w2_sb")
            eye_sb = sb.tile([12, 8], BF16, tag="eye", name="eye_sb")
            y0_sb = sb.tile([12, W], BF16, tag="y0", name="y0_sb")
            st_w1 = sb.tile([128, 100], F32, tag="sw1", name="st_w1")
            st_w2 = sb.tile([101, 8 * nu], F32, tag="sw2", name="st_w2")
            st_eye = sb.tile([12, 8], F32, tag="sey", name="st_eye")
            st_y0 = sb.tile([12, W], F32, tag="sy0", name="st_y0")
            yr = sb.tile([128, 2048], BF16, tag="yr", name="yr")
            h_bufs = [
                sb.tile([101, 1024], BF16, tag=f"h{i}", name=f"h{i}")
                for i in range(HB)
            ]
            y_ps = ps.tile([128, 2048], F32, tag="y", name="y_ps")
            a_bufs = [
                ps.tile([128, 1024], F32, tag=f"a{i}", name=f"a{i}") for i in range(2)
            ]

            nc.sync.dma_start(out=st_w1[:, :], in_=w1rep_d[:, :])
            nc.sync.dma_start(out=b1_sb[:, :], in_=b1bd_d[:, :])
            nc.sync.dma_start(out=st_w2[:, :], in_=w2f_d[:, :])
            nc.sync.dma_start(out=st_eye[:, :], in_=eye12_d[:, :])
            nc.sync.dma_start(out=st_y0[:, :], in_=y0pre_d[:, :])
            nc.vector.tensor_copy(out=w1_sb[:, :], in_=st_w1[:, :])
            nc.vector.tensor_copy(out=w2_sb[:, :], in_=st_w2[:, :])
            nc.vector.tensor_copy(out=eye_sb[:, :], in_=st_eye[:, :])
            nc.vector.tensor_copy(out=y0_sb[:, :], in_=st_y0[:, :])

            # h row 100 is a constant-1 bias row (so mm2 adds dt*b2 via
            # lhsT row 100). memset can't write BF16 reliably and engine
            # partition bases must be 32-aligned, so stage rows 96-101 in
            # F32 and copy; rows 96-99 get overwritten by every tanh.
            ones_st = sb.tile([101, 1024], F32, tag="one", name="ones_st")
            nc.vector.memset(ones_st[96:101, :], 1.0)
            for i in range(HB):
                nc.vector.tensor_copy(
                    out=h_bufs[i][96:101, :], in_=ones_st[96:101, :]
                )

            # init: y_psum[chunk] = y0hi + y0mid + y0lo via identity matmul
            for c in range(16):
                j, b = c % 4, c // 4
                nc.tensor.matmul(
                    y_ps[32 * j : 32 * j + 8, 512 * b : 512 * b + 512],
                    lhsT=eye_sb[:, :],
                    rhs=y0_sb[:, 512 * c : 512 * c + 512],
                    start=True,
                    stop=True,
                    tile_position=(0, 32 * j),
                )

            def recirc(j, ph, eng=None):
                # PSUM y (fp32) -> SBUF yr (bf16) for the next step's mm1;
                # DMA/gpsimd can't read PSUM so this is DVE work, except
                # strip 3's pieces which ride in ACT's phase-boundary stall
                # gaps (their consumers have the most slack). Split per
                # (strip, phase) so it overlaps compute of the other phase.
                cs = slice(1024 * ph, 1024 * ph + 1024)
                dst = yr[32 * j : 32 * j + 8, cs]
                src = y_ps[32 * j : 32 * j + 8, cs]
                if eng == "act":
                    nc.scalar.copy(out=dst, in_=src)
                else:
                    nc.vector.tensor_copy(out=dst, in_=src)

            def mm1(r, u):
                # global round r: step r//8, local round k=r%8 covers local
                # chunks {2k, 2k+1}; chunk c -> strip j=c%4, bank b=c//4.
                # Consecutive rounds use disjoint a tiles so the tanh WAR
                # chain spans 2 rounds (slack for sem hops).
                c = 2 * (r % 8) + u
                j, b = c % 4, c // 4
                nc.tensor.matmul(
                    a_bufs[r % 2][0:100, 512 * u : 512 * u + 512],
                    lhsT=w1_sb[32 * j : 32 * j + 8, :],
                    rhs=yr[32 * j : 32 * j + 8, 512 * b : 512 * b + 512],
                    start=True,
                    stop=True,
                    tile_position=(32 * j, 0),
                )

            def tanh(r):
                h = h_bufs[r % HB]
                nc.scalar.activation(
                    h[0:100, :],
                    a_bufs[r % 2][0:100, :],
                    TANH,
                    bias=b1_sb[:, :],
                )

            def mm2(s, c, di):
                j, b = c % 4, c // 4
                h = h_bufs[(8 * s + c // 2) % HB]
                hs = h[:, 512 * (c % 2) : 512 * (c % 2) + 512]
                nc.tensor.matmul(
                    y_ps[32 * j : 32 * j + 8, 512 * b : 512 * b + 512],
                    lhsT=w2_sb[:, 8 * di : 8 * di + 8],
                    rhs=hs,
                    start=False,
                    stop=True,
                    tile_position=(0, 32 * j),
                    skip_group_check=True,
                )

            def outdma(s, j, ph):
                # out[s] col 512*(4b+j)+i <- yr[2AB+c at 32j, 512b+i]
                nc.sync.dma_start(
                    out=out_d[s].rearrange(
                        "p (b j2 i) -> p b j2 i", b=4, j2=4, i=512
                    )[:, 2 * ph : 2 * ph + 2, j],
                    in_=yr[32 * j : 32 * j + 4, 1024 * ph : 1024 * ph + 1024]
                    .rearrange("p (b i) -> p b i", b=2, i=512),
                )

            for j in range(4):
                recirc(j, 0)
                recirc(j, 1)

            # Software-pipelined rounds of 2 chunks: even rounds hit strips
            # {0,1}, odd {2,3}; consecutive rounds use disjoint a tiles so
            # the tanh->mm1 WAR chain spans 2 rounds (slack for sem hops).
            # mm2s batch 4-way (4 distinct col strips) after each odd
            # round's tanh; recirc/out DMA per phase overlap 4+ rounds.
            R = steps * 8
            for r in range(R + 1):
                if r < R:
                    mm1(r, 0)
                    mm1(r, 1)
                if r >= 2 and (r - 1) % 8 % 2 == 1:
                    ro = r - 1  # odd round whose tanh is already emitted
                    s, k8 = ro // 8, ro % 8
                    di = d_idx[s]
                    for c in range(4 * (k8 // 2), 4 * (k8 // 2) + 4):
                        mm2(s, c, di)
                    if k8 == 3:
                        for j in range(4):
                            recirc(j, 0, eng="act" if j == 3 else None)
                            outdma(s, j, 0)
                    elif k8 == 7:
                        for j in range(4):
                            recirc(j, 1, eng="act" if j == 3 else None)
                            outdma(s, j, 1)
                if r < R:
                    tanh(r)
    nc.compile()
    return nc


def _prep(y0, t, w1, b1, w2, b2, ncores):
    B = y0.shape[0]
    steps = t.shape[0] - 1
    N = B // ncores
    dts = (t[1:] - t[:-1]).astype(np.float32)
    uniq, inv = np.unique(dts, return_inverse=True)
    nu = len(uniq)
    w1hi = _rbf(w1)
    w1lo = _rbf((w1 - w1hi).astype(np.float32))
    w1bd = np.zeros((8, 100), np.float32)
    w1bd[0:2, 0:50] = w1hi.T
    w1bd[2:4, 50:100] = w1hi.T
    w1bd[4:6, 0:50] = w1lo.T
    w1bd[6:8, 50:100] = w1lo.T
    w1rep = np.zeros((128, 100), np.float32)
    for j in range(4):
        w1rep[32 * j : 32 * j + 8] = w1bd
    b1bd = np.concatenate([b1, b1]).astype(np.float32).reshape(100, 1)
    w2f = np.zeros((101, 8 * nu), np.float32)
    for d in range(nu):
        dw2 = (uniq[d] * w2).astype(np.float32)
        db2 = (uniq[d] * b2).astype(np.float32)
        hi2 = _rbf(dw2)
        bhi = _rbf(db2)
        for dup in (0, 1):
            o = 8 * d + 4 * dup
            w2f[0:50, o : o + 2] = hi2.T
            w2f[50:100, o + 2 : o + 4] = hi2.T
        w2f[100, 8 * d : 8 * d + 8] = [bhi[0], bhi[1], bhi[0], bhi[1]] * 2
    eye12 = np.zeros((12, 8), np.float32)
    for r in range(12):
        for m in range(8):
            if r % 4 == m % 4:
                eye12[r, m] = 1.0
    y0hi = _rbf(y0)
    y0mid = _rbf((y0 - y0hi).astype(np.float32))
    y0lo = _rbf((y0 - y0hi - y0mid).astype(np.float32))
    in_maps = []
    for k in range(ncores):
        yk = np.empty((12, N // 2), np.float32)
        for src, base in ((y0hi, 0), (y0mid, 4), (y0lo, 8)):
            # row 2*AB+c, col 512*chunk+i = src[kN + 1024*chunk + 512*AB + i, c]
            blk = src[k * N : (k + 1) * N].reshape(16, 2, 512, 2)  # chunk,AB,i,c
            yk[base : base + 4] = (
                blk.transpose(1, 3, 0, 2).reshape(4, N // 2)
            )
        in_maps.append(
            {
                "w1rep": w1rep,
                "b1bd": b1bd,
                "w2f": w2f,
                "eye12": eye12,
                "y0pre": yk,
            }
        )
    return nu, list(inv), steps, N, in_maps


def run(y0, t, w1, b1, w2, b2, ncores=NCORES, steps_override=None, trace=False):
    y0 = np.ascontiguousarray(y0, dtype=np.float32)
    nu, inv, steps, N, in_maps = _prep(
        y0, np.asarray(t), np.asarray(w1), np.asarray(b1), np.asarray(w2),
        np.asarray(b2), ncores,
    )
    if steps_override is not None:
        steps = steps_override
    nc = _build(nu, inv, steps, ncores)
    res = bass_utils.run_bass_kernel_spmd(
        nc, in_maps, list(range(ncores)), trace=trace
    )
    B = y0.shape[0]
    out = np.empty((steps + 1, B, 2), np.float32)
    out[0] = y0
    for k in range(ncores):
        v = np.asarray(res.results[k]["out"]).astype(np.float32)
        v = v.reshape(steps, 2, 2, 16, 512)  # s, AB, c, chunk, i
        out[1:, k * N : (k + 1) * N, :] = (
            v.transpose(0, 3, 1, 4, 2).reshape(steps, N, 2)
        )
    return out, res


def kernel(**inputs):
    out, _ = run(
        inputs["y0"], inputs["t"], inputs["w1"], inputs["b1"], inputs["w2"],
        inputs["b2"],
    )
    return out
